# revision 13
# baseline (speedup 1.0000x reference)
"""Trainium2 Bass kernel for the capsule-routing layer.

Math (derived from the reference):
  u_hat[b,i,j,k] = sum_d x[b,j,d] W[d, i*32+k]   (never materialized!)
  iter t: c = softmax_i(b_logits); s[i,k] = sum_j c[i,j] u_hat[i,j,k]
          o = s / sqrt(sum_k s^2 + eps); b_logits[i,j] = sum_k o[i,k] u_hat[i,j,k]
Substituting u_hat = x @ W everywhere:
  y[i,d]   = sum_j c[i,j] x[j,d]            (small matmul, K=1024)
  s[i,k]   = sum_d y[i,d] W[d, i*32+k]      (block-diagonal of y @ W)
  wtil[d,i]= sum_k W[d, i*32+k] o[i,k]      (W @ block-diag(o))
  b[i,j]   = sum_d x[j,d] wtil[d,i]         (small matmul, K=256)
This removes the 34-GFLOP u_hat product entirely (~7.6x FLOP reduction).

Sharding: data-parallel, 8 batches per core; batches processed in groups of
4 stacked on SBUF partitions (partition p = 32*b + i).
"""

import numpy as np

try:
    import concourse.bass as bass
except ImportError:  # path fallback for bare environments
    import sys

    sys.path.insert(0, "/opt/trn_rl_repo")
    import concourse.bass as bass

from contextlib import ExitStack

import concourse.bacc as bacc
import concourse.tile as tile
from concourse import mybir
from concourse.bass_utils import run_bass_kernel_spmd

F32 = mybir.dt.float32
F32R = mybir.dt.float32r
BF16 = mybir.dt.bfloat16
AF = mybir.ActivationFunctionType
ALU = mybir.AluOpType

NUM_CAPS = 32
DIM_CAPS = 32
D_IN = 256  # feature dim (d)
N_IN = 1024  # input capsule count (j)
IK = NUM_CAPS * DIM_CAPS  # 1024 flattened (i,k)
B_TOTAL = 64
N_CORES = 8
B_PER_CORE = 8
GB = 4  # batches per partition-group
GROUPS = B_PER_CORE // GB  # 2
EPS = 1e-7
ROUTINGS = 3


def _r(ap):
    """Matmul operands are declared float32r; pass through."""
    return ap


def build_program():
    nc = bacc.Bacc("TRN2", target_bir_lowering=False, debug=False)

    x_b = nc.declare_dram_parameter("x_b", [B_PER_CORE, N_IN, D_IN], BF16, isOutput=False)
    x_d = nc.declare_dram_parameter("x_d", [B_PER_CORE, D_IN, N_IN], BF16, isOutput=False)
    w_a = nc.declare_dram_parameter("w_a", [D_IN, IK], F32R, isOutput=False)
    w_t = nc.declare_dram_parameter("w_t", [IK, D_IN], BF16, isOutput=False)
    mask_d = nc.declare_dram_parameter("mask", [128, IK], F32, isOutput=False)
    ident_d = nc.declare_dram_parameter("ident", [128, 128], F32, isOutput=False)
    out_d = nc.declare_dram_parameter("out", [GROUPS, 128, DIM_CAPS], F32, isOutput=True)

    with ExitStack() as ctx:
        tc = ctx.enter_context(tile.TileContext(nc))
        singles = ctx.enter_context(tc.tile_pool(name="singles", bufs=1))
        xpool = ctx.enter_context(tc.tile_pool(name="xpool", bufs=6))
        work = ctx.enter_context(tc.tile_pool(name="work", bufs=2))
        psum = ctx.enter_context(tc.tile_pool(name="ps", bufs=1, space="PSUM"))

        # ---- static tensors ----
        w_a_sb = singles.tile([128, 2, IK], F32R)  # [d%128, d//128, (ik)]
        nc.sync.dma_start(out=w_a_sb[:, :, :], in_=w_a[:, :].rearrange("(c p) n -> p c n", p=128))
        w_t_sb = singles.tile([128, 8, D_IN], BF16)  # [(ik)%128, (ik)//128, d]
        nc.sync.dma_start(out=w_t_sb[:, :, :], in_=w_t[:, :].rearrange("(c p) n -> p c n", p=128))
        mask_sb = singles.tile([128, IK], F32)
        nc.sync.dma_start(out=mask_sb[:, :], in_=mask_d[:, :])
        ident_sb = singles.tile([128, 128], F32)
        nc.sync.dma_start(out=ident_sb[:, :], in_=ident_d[:, :])
        cu_sb = singles.tile([128, NUM_CAPS], BF16)
        nc.vector.memset(cu_sb[:, :], 1.0 / NUM_CAPS)
        eps_sb = singles.tile([128, 1], F32)
        nc.vector.memset(eps_sb[:, :], EPS)

        for g in range(GROUPS):
            # ---- load this group's x in both layouts ----
            xb_t = []
            xd_t = []
            for b in range(GB):
                bb = g * GB + b
                xb = xpool.tile([128, 8, D_IN], BF16, tag="xb")  # [j%128, j//128, d]
                nc.sync.dma_start(out=xb[:, :, :], in_=x_b[bb].rearrange("(c p) n -> p c n", p=128))
                xb_t.append(xb)
                xd = xpool.tile([128, 2, N_IN], BF16, tag="xd")  # [d%128, d//128, j]
                nc.sync.dma_start(out=xd[:, :, :], in_=x_d[bb].rearrange("(c p) n -> p c n", p=128))
                xd_t.append(xd)

            cT_sb = None  # [j%128, j//128, (4b,32i)] softmax'd coupling coeffs
            for it in range(ROUTINGS):
                last = it == ROUTINGS - 1

                # ---- y-MM: y[b,i,d] = sum_j c[b,i,j] x[b,j,d] ----
                y4_ps = psum.tile([128, D_IN], F32, tag="m32")  # [(4b,32i), d]
                for jc in range(8):
                    for b in range(GB):
                        lhsT = cu_sb[:, :] if it == 0 else cT_sb[:, jc, 32 * b : 32 * b + 32]
                        nc.tensor.matmul(
                            y4_ps[32 * b : 32 * b + 32, :],
                            _r(lhsT),
                            _r(xb_t[b][:, jc, :]),
                            start=(jc == 0),
                            stop=(jc == 7),
                            tile_position=(0, 32 * b),
                            skip_group_check=True,
                        )

                # evacuate + transpose y -> [d, (4b,32i)]
                y4_sb = work.tile([128, D_IN], F32, tag="y4sb")
                nc.scalar.copy(y4_sb[:, :], y4_ps[:, :])
                yT_ps = psum.tile([128, 2, 128], F32, tag="tp2")
                for t in range(2):
                    nc.tensor.transpose(yT_ps[:, t, :], y4_sb[:, 128 * t : 128 * t + 128], ident_sb[:, :])
                yT_sb = work.tile([128, 2, 128], F32R, tag="yTsb")
                nc.vector.tensor_copy(yT_sb[:, :, :], yT_ps[:, :, :])

                # ---- s-MM (cross): s_cross[(b,i),(i',k)] = sum_d y[b,i,d] W[d,(i'k)] ----
                sc_ps = psum.tile([128, IK], F32, tag="big", bufs=2)
                for dc in range(2):
                    for nh in range(2):
                        nc.tensor.matmul(
                            sc_ps[:, 512 * nh : 512 * nh + 512],
                            _r(yT_sb[:, dc, :]),
                            _r(w_a_sb[:, dc, 512 * nh : 512 * nh + 512]),
                            start=(dc == 0),
                            stop=(dc == 1),
                            skip_group_check=True,
                        )

                # ---- mask to the diagonal blocks (s values), evacuating PSUM ----
                if last:
                    m4_sb = work.tile([128, IK], F32, tag="m4f")
                else:
                    m4_sb = work.tile([128, IK], BF16, tag="m4")
                nc.vector.tensor_mul(m4_sb[:, :], sc_ps[:, :], mask_sb[:, :])

                if last:
                    # compact s[(b,i), k] = sum_{i'} masked[(b,i), (i',k)]
                    s4c = work.tile([128, DIM_CAPS], F32, tag="s4c")
                    nc.vector.tensor_reduce(
                        s4c[:, :],
                        m4_sb[:, :].rearrange("p (i k) -> p k i", i=NUM_CAPS),
                        axis=mybir.AxisListType.X,
                        op=ALU.add,
                    )
                    sq_s = work.tile([128, DIM_CAPS], F32, tag="sqs")
                    nsq = work.tile([128, 1], F32, tag="nsq")
                    nc.scalar.activation(sq_s[:, :], s4c[:, :], AF.Square, accum_out=nsq[:, :])
                    sn = work.tile([128, 1], F32, tag="sn")
                    nc.scalar.activation(sn[:, :], nsq[:, :], AF.Sqrt, bias=eps_sb[:, :])
                    rn = work.tile([128, 1], F32, tag="rn")
                    nc.vector.reciprocal(rn[:, :], sn[:, :])
                    o_out = work.tile([128, DIM_CAPS], F32, tag="oout")
                    nc.vector.tensor_scalar(o_out[:, :], s4c[:, :], rn[:, :], None, ALU.mult)
                    nc.sync.dma_start(out=out_d[g], in_=o_out[:, :])
                    continue

                # ---- squash norm from masked cross (sum of squares over free dim) ----
                sq_scr = work.tile([128, IK], BF16, tag="scr")
                nsq4 = work.tile([128, 1], F32, tag="nsq4")
                nc.scalar.activation(sq_scr[:, :], m4_sb[:, :], AF.Square, accum_out=nsq4[:, :])
                sn4 = work.tile([128, 1], F32, tag="sn4")
                nc.scalar.activation(sn4[:, :], nsq4[:, :], AF.Sqrt, bias=eps_sb[:, :])
                rn4 = work.tile([128, 1], F32, tag="rn4")
                nc.vector.reciprocal(rn4[:, :], sn4[:, :])

                # ---- O = transpose(masked s) -> [(ik), (4b,32i)] via DMA xbar ----
                o_sb = work.tile([128, 8, 128], BF16, tag="osb")
                nc.sync.dma_start_transpose(o_sb[:, :, :], m4_sb[:, :])

                # ---- wtil-MM: wT[(b,i), d] = sum_(ik) O[(ik),(b,i)] WT[(ik), d] ----
                wT_ps = psum.tile([128, D_IN], F32, tag="m32")
                for ikc in range(8):
                    for b in range(GB):
                        nc.tensor.matmul(
                            wT_ps[32 * b : 32 * b + 32, :],
                            _r(o_sb[:, ikc, 32 * b : 32 * b + 32]),
                            _r(w_t_sb[:, ikc, :]),
                            start=(ikc == 0),
                            stop=(ikc == 7),
                            tile_position=(0, 32 * b),
                            skip_group_check=True,
                        )
                # evacuate with the squash scale (o = s * rn) folded in
                wT_sb = work.tile([128, D_IN], F32, tag="wTsb")
                nc.vector.tensor_scalar(wT_sb[:, :], wT_ps[:, :], rn4[:, :], None, ALU.mult)

                # transpose wtil -> [d, (4b,32i)]
                wt_ps = psum.tile([128, 2, 128], F32, tag="tp2")
                for t in range(2):
                    nc.tensor.transpose(wt_ps[:, t, :], wT_sb[:, 128 * t : 128 * t + 128], ident_sb[:, :])
                wt_sb = work.tile([128, 2, 128], BF16, tag="wtsb")
                nc.vector.tensor_copy(wt_sb[:, :, :], wt_ps[:, :, :])

                # ---- b-MM: blogit[(b,i), j] = sum_d wtil[d,(b,i)] x[b][d, j] ----
                b4_ps = psum.tile([128, N_IN], F32, tag="big", bufs=2)
                for dc in range(2):
                    for jh in range(2):
                        for b in range(GB):
                            nc.tensor.matmul(
                                b4_ps[32 * b : 32 * b + 32, 512 * jh : 512 * jh + 512],
                                _r(wt_sb[:, dc, 32 * b : 32 * b + 32]),
                                _r(xd_t[b][:, dc, 512 * jh : 512 * jh + 512]),
                                start=(dc == 0),
                                stop=(dc == 1),
                                tile_position=(0, 32 * b),
                                skip_group_check=True,
                            )

                # ---- softmax over capsules i (partition blocks of 32) ----
                e4_sb = work.tile([128, N_IN], BF16, tag="e4")
                nc.scalar.activation(e4_sb[:, :], b4_ps[:, :], AF.Exp)
                eT_sb = work.tile([128, 8, 128], BF16, tag="eT")
                nc.sync.dma_start_transpose(eT_sb[:, :, :], e4_sb[:, :])
                zT_sb = work.tile([128, 8, GB], F32, tag="zT")
                nc.vector.tensor_reduce(
                    zT_sb[:, :, :],
                    eT_sb[:, :, :].rearrange("p c (b i) -> p c b i", b=GB),
                    axis=mybir.AxisListType.X,
                    op=ALU.add,
                )
                rz_sb = work.tile([128, 8, GB], F32, tag="rz")
                nc.vector.reciprocal(rz_sb[:, :, :], zT_sb[:, :, :])
                cT_sb = work.tile([128, 8, 128], BF16, tag="cT")
                nc.vector.tensor_tensor(
                    cT_sb[:, :, :].rearrange("p c (b i) -> p c b i", b=GB),
                    eT_sb[:, :, :].rearrange("p c (b i) -> p c b i", b=GB),
                    rz_sb[:, :, :].unsqueeze(3).broadcast_to([128, 8, GB, NUM_CAPS]),
                    ALU.mult,
                )

    nc.compile()
    return nc


def _host_inputs(x, W):
    import ml_dtypes

    bf16 = ml_dtypes.bfloat16
    x = np.ascontiguousarray(np.asarray(x, dtype=np.float32))
    W = np.ascontiguousarray(np.asarray(W, dtype=np.float32)).reshape(D_IN, IK)
    xT = np.ascontiguousarray(x.transpose(0, 2, 1)).astype(bf16)
    WT = np.ascontiguousarray(W.T).astype(bf16)
    x = x.astype(bf16)
    q = np.arange(IK)
    p = np.arange(128)
    mask = (q[None, :] // DIM_CAPS == p[:, None] % NUM_CAPS).astype(np.float32)
    ident = np.eye(128, dtype=np.float32)
    return x, xT, W, WT, mask, ident


_prog_cache = {}


def _get_program():
    if "nc" not in _prog_cache:
        _prog_cache["nc"] = build_program()
    return _prog_cache["nc"]


def kernel(x, W):
    x, xT, W, WT, mask, ident = _host_inputs(x, W)
    nc = _get_program()
    in_maps = []
    for c in range(N_CORES):
        sl = slice(c * B_PER_CORE, (c + 1) * B_PER_CORE)
        in_maps.append(
            {
                "x_b": x[sl],
                "x_d": xT[sl],
                "w_a": W,
                "w_t": WT,
                "mask": mask,
                "ident": ident,
            }
        )
    res = run_bass_kernel_spmd(nc, in_maps, core_ids=list(range(N_CORES)))
    out = np.empty((B_TOTAL, NUM_CAPS, DIM_CAPS), np.float32)
    for c in range(N_CORES):
        o = res.results[c]["out"]  # [GROUPS, 128, 32]; partition p = 32*b + i
        out[c * B_PER_CORE : (c + 1) * B_PER_CORE] = o.reshape(B_PER_CORE, NUM_CAPS, DIM_CAPS)
    return out
